# revision 10
# baseline (speedup 1.0000x reference)
"""Trainium2 Bass kernel for nn_HeatmapLayer: separable Gaussian heatmaps.

Reference math (per batch b, class c):
    mx = labels[b, 2c] * H ; my = labels[b, 2c+1] * W          (H = W = 384)
    sigma = H * exp(log_weight)
    dx2[h] = (h - mx)^2 / sigma        ; normalized by its min over h
    dy2[w] = (w - my)^2 / (20 * sigma) ; normalized by its min over w
    out[b,c,h,w] = exp(-0.5*(dx2[h] + dy2[w])) = ex[h] * ey[w]

Each (b,c) heatmap is a rank-1 outer product of two 384-length
profiles.  Pure data parallel over batch: 2 batches = 12 (b,c)
pairs per core; per-core output 12*384*384*4B = 7.08 MB, so the
per-core HBM write roofline (~358 GB/s) is ~19.8 us.  The kernel is
a latency problem: a fixed ~6.2us NEFF prologue + input-DMA
completion (~9us) gate the first output DMA; after that the write
stream runs at the HBM cap.

v6 structure (v4 baseline ~41us, v5 ~39.2us):

  * One [12, 2, 384] profile tile (block 0 = x side, block 1 = y
    side), built by a mostly-DVE chain (shift, square, min-reduce,
    scale, min-subtract via stride-0 broadcast APs) and ONE ACT Exp.
  * ey_p replicated to 128 partitions by the otherwise-idle PE with
    a selection-matrix matmul: psum = sel_p[12,128]^T @ ey[12,384],
    sel_p[k,m] = (k==p).  fp32r streams at 1 cycle/row; no
    SBUF-gather DMA on the critical path.
  * x-profiles PE-transposed to pt[par, r, pair] (= ex[3par+r]) in
    PSUM; the final multiplies read both PSUM operands directly.
  * 3 chunk multiplies per pair (DVE/ACT split; pair 0 all-DVE),
    writing the staged [128, 3, 384] tile; DRAM rows 3*par+r are
    contiguous per partition -> 4608B DMA descriptors, one 576KB
    HWDGE DMA per pair on the sync queue (pair 0 split 1+2 chunks
    to start the stream earlier).
  * A dummy ACT op forces the 1283ns activation-table load into the
    prologue/input-DMA window.

x is only used for its shape; it is never transferred to the device.
"""

import numpy as np
from contextlib import ExitStack

import concourse.bacc as bacc
import concourse.bass as bass
import concourse.tile as tile
from concourse import mybir
from concourse.bass_utils import run_bass_kernel_spmd
from concourse.masks import make_identity

B, CH, H, W = 16, 3, 384, 384
NCLS = 6
N_CORES = 8
BPC = B // N_CORES            # batches per core = 2
PAIRS = BPC * NCLS            # (b,c) pairs per core = 12
P = 128
RPP = H // P                  # DRAM rows per partition = 3
LN_H = float(np.log(H))
F32 = mybir.dt.float32
F32R = mybir.dt.float32r
AF = mybir.ActivationFunctionType

# invk[:, 0] = inv_s/2 = exp(-logw - lnH + ln(1/2));  invk[:, 1] = inv_s/40
BIAS_KX = float(np.log(0.5)) - LN_H

# engine for the 36 final multiplies, by flat index (p*RPP + r);
# pair 0 all-DVE (ACT is still finishing the profile Exp).
MULT_ENGINE = "vvv" + "vvs" * (PAIRS - 1)


def build_bass() -> bass.Bass:
    nc = bacc.Bacc("TRN2", target_bir_lowering=False, debug=False,
                   num_devices=N_CORES)
    labels = nc.dram_tensor("labels", [BPC, 2 * NCLS], F32,
                            kind="ExternalInput")
    logw = nc.dram_tensor("log_weight", [1, 1], F32, kind="ExternalInput")
    out = nc.dram_tensor("out", [PAIRS * H, W], F32, kind="ExternalOutput")

    with ExitStack() as ctx:
        tc = ctx.enter_context(tile.TileContext(nc))
        singles = ctx.enter_context(tc.tile_pool(name="singles", bufs=1))
        psum = ctx.enter_context(tc.tile_pool(name="psum", bufs=5,
                                              space="PSUM"))
        psumT = ctx.enter_context(tc.tile_pool(name="psumT", bufs=1,
                                               space="PSUM"))
        stage = ctx.enter_context(tc.tile_pool(name="stage", bufs=6))

        # ---- constants (no input deps; overlap prologue/input DMAs) -----
        ones = singles.tile([1, 1], F32)
        nc.vector.memset(ones, 1.0)
        ident = singles.tile([PAIRS, PAIRS], F32)
        make_identity(nc, ident)
        iog = singles.tile([PAIRS, W], F32)
        nc.gpsimd.iota(iog, pattern=[[1, W]], base=0, channel_multiplier=0,
                       allow_small_or_imprecise_dtypes=True)
        # sel[k, p, m] = 1.0 if k == p else 0.0  (PE broadcast weights)
        sel = singles.tile([PAIRS, PAIRS, P], F32)
        nc.gpsimd.memset(sel, 1.0)
        nc.gpsimd.affine_select(
            out=sel, in_=sel, compare_op=mybir.AluOpType.is_equal,
            fill=0.0, base=0, channel_multiplier=1,
            pattern=[[-1, PAIRS], [0, P]],
        )
        # dummy ACT op: forces the 1283ns Exp-table load to run early
        warm = singles.tile([1, 1], F32)
        nc.scalar.activation(out=warm, in_=ones, func=AF.Exp,
                             bias=0.0, scale=0.0)

        # ---- inputs ------------------------------------------------------
        lab = singles.tile([PAIRS, 2], F32)     # row p: (mx_p, my_p)/H
        nc.sync.dma_start(
            out=lab,
            in_=labels[:, :].rearrange("b (q two) -> (b q) two", two=2),
        )
        lw = singles.tile([PAIRS, 1], F32)
        nc.scalar.dma_start(out=lw, in_=logw[:, :].to_broadcast((PAIRS, 1)))

        # ---- invk[12, 2]: per-block exponent scales ---------------------
        bk = singles.tile([PAIRS, 1], F32)
        nc.gpsimd.memset(bk, BIAS_KX)
        invk = singles.tile([PAIRS, 2], F32)
        nc.scalar.activation(out=invk[:, 0:1], in_=lw, func=AF.Exp,
                             bias=bk, scale=-1.0)
        nc.vector.tensor_scalar_mul(out=invk[:, 1:2], in0=invk[:, 0:1],
                                    scalar1=0.05)

        # ---- profiles: exy[p, 0, h] = ex_p(h), exy[p, 1, w] = ey_p(w) ---
        negm = singles.tile([PAIRS, 2], F32)
        nc.vector.tensor_scalar_mul(out=negm, in0=lab, scalar1=-float(H))
        iov = iog[:, :]
        io2 = bass.AP(tensor=iov.tensor, offset=iov.offset,
                      ap=[iov.ap[0], [0, 2], iov.ap[1]])
        d = singles.tile([PAIRS, 2, W], F32)
        nc.vector.tensor_tensor(out=d, in0=io2,
                                in1=negm[:, :].to_broadcast((PAIRS, 2, W)),
                                op=mybir.AluOpType.add)
        sq = singles.tile([PAIRS, 2, W], F32)
        nc.vector.tensor_mul(out=sq, in0=d, in1=d)
        mn = singles.tile([PAIRS, 2], F32)
        nc.vector.tensor_reduce(out=mn, in_=sq, axis=mybir.AxisListType.X,
                                op=mybir.AluOpType.min)
        mnk = singles.tile([PAIRS, 2], F32)
        nc.vector.tensor_mul(out=mnk, in0=mn, in1=invk)
        t = singles.tile([PAIRS, 2, W], F32)
        nc.vector.tensor_tensor(out=t, in0=sq,
                                in1=invk[:, :].to_broadcast((PAIRS, 2, W)),
                                op=mybir.AluOpType.mult)
        t2 = singles.tile([PAIRS, 2, W], F32)
        nc.vector.tensor_tensor(out=t2, in0=t,
                                in1=mnk[:, :].to_broadcast((PAIRS, 2, W)),
                                op=mybir.AluOpType.subtract)
        exy = singles.tile([PAIRS, 2, W], F32)
        nc.scalar.activation(out=exy, in_=t2, func=AF.Exp,
                             bias=0.0, scale=-1.0)

        # ---- x-profiles transposed to pt[par, r, pair] via PE -----------
        # pt[par, r, p] = ex[p, 3*par + r]
        exv = exy[:, 0, :].rearrange("p (k r) -> p r k", r=RPP)
        pt = psumT.tile([P, RPP, PAIRS], F32)
        for r in range(RPP):
            nc.tensor.transpose(pt[:, r, :], exv[:, r, :], ident)
        ext = singles.tile([P, RPP, PAIRS], F32)
        nc.vector.tensor_copy(out=ext, in_=pt)

        # ---- main loop ---------------------------------------------------
        for p in range(PAIRS):
            ps = psum.tile([P, W], F32)
            nc.tensor.matmul(ps, sel[:, p, :], exy[:, 1, :],
                             start=True, stop=True)
            st = stage.tile([P, RPP, W], F32)
            for r in range(RPP):
                scal = ext[:, r, p:p + 1]
                if MULT_ENGINE[p * RPP + r] == "v":
                    nc.vector.tensor_scalar_mul(out=st[:, r, :], in0=ps,
                                                scalar1=scal)
                else:
                    nc.scalar.mul(out=st[:, r, :], in_=ps, mul=scal)
            # partition par holds DRAM rows 3*par..3*par+2 of pair p:
            # one contiguous 4608B descriptor per partition.
            odst = out[p * H:(p + 1) * H, :].rearrange(
                "(par r) w -> par r w", par=P)
            if p == 0:
                # split: start the write stream as soon as chunk 0 exists
                nc.sync.dma_start(out=odst[:, 0:1, :], in_=st[:, 0:1, :])
                nc.sync.dma_start(out=odst[:, 1:, :], in_=st[:, 1:, :])
            else:
                nc.sync.dma_start(out=odst, in_=st)
    nc.finalize()
    return nc


LAST_RESULTS = None  # BassKernelResults of the most recent kernel() call


def kernel(x: np.ndarray, labels: np.ndarray,
           log_weight: np.ndarray, **run_kwargs) -> np.ndarray:
    global LAST_RESULTS
    del x  # only its (hardcoded) shape matters
    nc = build_bass()
    labels = np.ascontiguousarray(labels, dtype=np.float32)
    lw = np.ascontiguousarray(log_weight, dtype=np.float32).reshape(1, 1)
    in_maps = [
        {"labels": labels[i * BPC:(i + 1) * BPC], "log_weight": lw}
        for i in range(N_CORES)
    ]
    res = run_bass_kernel_spmd(nc, in_maps, core_ids=list(range(N_CORES)),
                               **run_kwargs)
    LAST_RESULTS = res
    outs = [r["out"].reshape(BPC, NCLS, H, W) for r in res.results]
    return np.concatenate(outs, axis=0)


if __name__ == "__main__":
    rng = np.random.default_rng(0)
    x = rng.standard_normal((B, CH, H, W), dtype=np.float32)
    labels = rng.random((B, 2 * NCLS), dtype=np.float32)
    lw = rng.random((1, 1, 1, 1), dtype=np.float32)
    y = kernel(x=x, labels=labels, log_weight=lw)
    print(y.shape, y.dtype, y.min(), y.max())


# revision 11
# speedup vs baseline: 1.1524x; 1.1524x over previous
"""Trainium2 Bass kernel for nn_HeatmapLayer: separable Gaussian heatmaps.

Reference math (per batch b, class c):
    mx = labels[b, 2c] * H ; my = labels[b, 2c+1] * W          (H = W = 384)
    sigma = H * exp(log_weight)
    dx2[h] = (h - mx)^2 / sigma        ; normalized by its min over h
    dy2[w] = (w - my)^2 / (20 * sigma) ; normalized by its min over w
    out[b,c,h,w] = exp(-0.5*(dx2[h] + dy2[w])) = ex[h] * ey[w]

Each (b,c) heatmap is a rank-1 outer product of two 384-length
profiles.  Pure data parallel over batch: 12 (b,c) pairs per core;
per-core output 7.08 MB, HBM-write roofline ~19.8us at 358 GB/s.
The kernel is latency-shaped: a fixed ~6.2us NEFF prologue + input
DMA completion (~2.5us) gate the first output DMA; the write stream
then runs at the HBM cap.  Optimization = start the stream ASAP.

v7 structure (v5 39.2us measured):

  * log_weight enters as a 1-descriptor DMA and is broadcast to 12
    partitions by the PE (ones[1,12]^T @ lw[1,1]) - no 12-descriptor
    broadcast DMA on the critical path.
  * Profiles per side (x block 0, y block 1 of sq[12,2,384]): the
    sqrt-scale is folded into ACT Square's scale/bias, one DVE
    min-reduce per side, one ACT Exp per side (384-wide ops; 12
    partitions x 384 free - never 768-wide, which doubles op time).
  * ey_p replicated to 128 partitions by the otherwise-idle PE with
    a bf16 selection-matrix matmul (sel_p[k,m] = (k==p), built by
    gpsimd off the critical path): psum = sel_p^T @ ey_bf.  bf16
    streams 1 cycle/row (~0.4us) vs fp32's 2 half-rate passes
    (~1.3us); bf16 rounding of ey costs ~4e-3 rel err vs the 2e-2
    gate.  exp_y writes the bf16 tile directly.
  * x-profiles PE-transposed (3 strided [12,128] transposes) to
    pt[par, r, pair] = ex[3par+r], one DVE copy to SBUF for the
    multiply scalars.
  * 3 chunk multiplies per pair read PSUM directly (DVE/ACT split),
    one 576KB DMA per pair alternating sync/scalar HWDGE queues;
    pair 0 is split 1+2 chunks to start the stream earliest.
  * A dummy ACT op forces the 1283ns Exp-table load into the
    prologue window (ACT's first real op otherwise pays it after
    the lw broadcast lands).

x is only used for its shape; it is never transferred to the device.
"""

import numpy as np
from contextlib import ExitStack

import concourse.bacc as bacc
import concourse.bass as bass
import concourse.tile as tile
from concourse import mybir
from concourse.bass_utils import run_bass_kernel_spmd
from concourse.masks import make_identity

B, CH, H, W = 16, 3, 384, 384
NCLS = 6
N_CORES = 8
BPC = B // N_CORES            # batches per core = 2
PAIRS = BPC * NCLS            # (b,c) pairs per core = 12
P = 128
RPP = H // P                  # DRAM rows per partition = 3
LN_H = float(np.log(H))
F32 = mybir.dt.float32
BF16 = mybir.dt.bfloat16
AF = mybir.ActivationFunctionType

# ssq_x = sqrt(inv_s/2) = exp(-0.5*logw - lnH/2 + ln(1/2)/2)
BIAS_SX = -LN_H / 2 + float(np.log(0.5)) / 2
RATIO_Y = float(np.sqrt(0.05))      # ssq_y / ssq_x = sqrt((1/40)/(1/2))

# engine for the 36 final multiplies, by flat index (p*RPP + r)
MULT_ENGINE = "vsv" + "vvs" * (PAIRS - 1)


def build_bass() -> bass.Bass:
    nc = bacc.Bacc("TRN2", target_bir_lowering=False, debug=False,
                   num_devices=N_CORES)
    labels = nc.dram_tensor("labels", [BPC, 2 * NCLS], F32,
                            kind="ExternalInput")
    logw = nc.dram_tensor("log_weight", [1, 1], F32, kind="ExternalInput")
    out = nc.dram_tensor("out", [PAIRS * H, W], F32, kind="ExternalOutput")

    with ExitStack() as ctx:
        tc = ctx.enter_context(tile.TileContext(nc))
        singles = ctx.enter_context(tc.tile_pool(name="singles", bufs=1))
        psum = ctx.enter_context(tc.tile_pool(name="psum", bufs=5,
                                              space="PSUM"))
        psumT = ctx.enter_context(tc.tile_pool(name="psumT", bufs=1,
                                               space="PSUM"))
        psumS = ctx.enter_context(tc.tile_pool(name="psumS", bufs=1,
                                               space="PSUM"))
        stage = ctx.enter_context(tc.tile_pool(name="stage", bufs=6))

        # ---- constants (no input deps; overlap prologue/input DMAs) -----
        ones = singles.tile([1, PAIRS], F32)
        nc.vector.memset(ones, 1.0)
        bx = singles.tile([PAIRS, 1], F32)
        nc.vector.memset(bx, BIAS_SX)
        iog = singles.tile([PAIRS, W], F32)
        nc.gpsimd.iota(iog, pattern=[[1, W]], base=0, channel_multiplier=0,
                       allow_small_or_imprecise_dtypes=True)
        ident = singles.tile([PAIRS, PAIRS], F32)
        make_identity(nc, ident)
        # sel[k, p, m] = 1.0 if k == p else 0.0  (bf16 PE broadcast weights)
        sel = singles.tile([PAIRS, PAIRS, P], BF16)
        nc.gpsimd.memset(sel, 1.0)
        nc.gpsimd.affine_select(
            out=sel, in_=sel, compare_op=mybir.AluOpType.is_equal,
            fill=0.0, base=0, channel_multiplier=1,
            pattern=[[-1, PAIRS], [0, P]],
        )
        # dummy ACT op: forces the 1283ns Exp-table load to run early
        warm = singles.tile([1, 1], F32)
        nc.scalar.activation(out=warm, in_=ones[:, 0:1], func=AF.Exp,
                             bias=0.0, scale=0.0)

        # ---- inputs ------------------------------------------------------
        lw0 = singles.tile([1, 1], F32)
        nc.sync.dma_start(out=lw0, in_=logw[:, :])
        lab = singles.tile([PAIRS, 2], F32)     # row p: (mx_p, my_p)/H
        nc.sync.dma_start(
            out=lab,
            in_=labels[:, :].rearrange("b (q two) -> (b q) two", two=2),
        )

        # ---- ssq[12, 2] = sqrt(k_i * inv_s) via PE broadcast of lw ------
        plw = psumS.tile([PAIRS, 1], F32)
        nc.tensor.matmul(plw, ones[:, :], lw0[:, :], start=True, stop=True)
        ssq = singles.tile([PAIRS, 2], F32)
        nc.scalar.activation(out=ssq[:, 0:1], in_=plw, func=AF.Exp,
                             bias=bx, scale=-0.5)
        nc.vector.tensor_scalar_mul(out=ssq[:, 1:2], in0=ssq[:, 0:1],
                                    scalar1=RATIO_Y)

        # ---- profiles: sq = (ssq*(w-m))^2 ; e = exp(mn - sq) per side ---
        negm = singles.tile([PAIRS, 2], F32)
        nc.vector.tensor_scalar_mul(out=negm, in0=lab, scalar1=-float(H))
        sb = singles.tile([PAIRS, 2], F32)
        nc.vector.tensor_mul(out=sb, in0=ssq, in1=negm)
        sq = singles.tile([PAIRS, 2, W], F32)
        mn = singles.tile([PAIRS, 2], F32)
        for i in range(2):
            nc.scalar.activation(out=sq[:, i, :], in_=iog, func=AF.Square,
                                 bias=sb[:, i:i + 1], scale=ssq[:, i:i + 1])
            nc.vector.tensor_reduce(out=mn[:, i:i + 1], in_=sq[:, i, :],
                                    axis=mybir.AxisListType.X,
                                    op=mybir.AluOpType.min)
        ex = singles.tile([PAIRS, W], F32)
        nc.scalar.activation(out=ex, in_=sq[:, 0, :], func=AF.Exp,
                             bias=mn[:, 0:1], scale=-1.0)
        ey_bf = singles.tile([PAIRS, W], BF16)
        nc.scalar.activation(out=ey_bf, in_=sq[:, 1, :], func=AF.Exp,
                             bias=mn[:, 1:2], scale=-1.0)

        # ---- x-profiles transposed to ext[par, r, pair] via PE ----------
        # ext[par, r, p] = ex[p, 3*par + r]
        exv = ex[:, :].rearrange("p (k r) -> p r k", r=RPP)
        pt = psumT.tile([P, RPP, PAIRS], F32)
        for r in range(RPP):
            nc.tensor.transpose(pt[:, r, :], exv[:, r, :], ident)
        ext = singles.tile([P, RPP, PAIRS], F32)
        nc.vector.tensor_copy(out=ext, in_=pt)

        # ---- main loop ---------------------------------------------------
        for p in range(PAIRS):
            ps = psum.tile([P, W], F32)
            nc.tensor.matmul(ps, sel[:, p, :], ey_bf[:, :],
                             start=True, stop=True)
            st = stage.tile([P, RPP, W], F32)
            for r in range(RPP):
                scal = ext[:, r, p:p + 1]
                if MULT_ENGINE[p * RPP + r] == "v":
                    nc.vector.tensor_scalar_mul(out=st[:, r, :], in0=ps,
                                                scalar1=scal)
                else:
                    nc.scalar.mul(out=st[:, r, :], in_=ps, mul=scal)
            # partition par holds DRAM rows 3*par..3*par+2 of pair p:
            # one contiguous 4608B descriptor per partition.
            odst = out[p * H:(p + 1) * H, :].rearrange(
                "(par r) w -> par r w", par=P)
            dma_eng = nc.sync if p % 2 == 0 else nc.scalar
            if p == 0:
                # split: start the write stream as soon as chunk 0 exists
                dma_eng.dma_start(out=odst[:, 0:1, :], in_=st[:, 0:1, :])
                dma_eng.dma_start(out=odst[:, 1:, :], in_=st[:, 1:, :])
            else:
                dma_eng.dma_start(out=odst, in_=st)
    nc.finalize()
    return nc


LAST_RESULTS = None  # BassKernelResults of the most recent kernel() call


def kernel(x: np.ndarray, labels: np.ndarray,
           log_weight: np.ndarray, **run_kwargs) -> np.ndarray:
    global LAST_RESULTS
    del x  # only its (hardcoded) shape matters
    nc = build_bass()
    labels = np.ascontiguousarray(labels, dtype=np.float32)
    lw = np.ascontiguousarray(log_weight, dtype=np.float32).reshape(1, 1)
    in_maps = [
        {"labels": labels[i * BPC:(i + 1) * BPC], "log_weight": lw}
        for i in range(N_CORES)
    ]
    res = run_bass_kernel_spmd(nc, in_maps, core_ids=list(range(N_CORES)),
                               **run_kwargs)
    LAST_RESULTS = res
    outs = [r["out"].reshape(BPC, NCLS, H, W) for r in res.results]
    return np.concatenate(outs, axis=0)


if __name__ == "__main__":
    rng = np.random.default_rng(0)
    x = rng.standard_normal((B, CH, H, W), dtype=np.float32)
    labels = rng.random((B, 2 * NCLS), dtype=np.float32)
    lw = rng.random((1, 1, 1, 1), dtype=np.float32)
    y = kernel(x=x, labels=labels, log_weight=lw)
    print(y.shape, y.dtype, y.min(), y.max())
